# revision 3
# baseline (speedup 1.0000x reference)
"""Distributed AttentionBlock kernel for 8 TRN2 NeuronCores.

Sharding: tensor-parallel over heads (16 heads -> 2 per core) for
qkv-projection + attention; per-batch AllToAll redistributes attention
output so each core computes the out-projection for a 512-token slice of
each batch; host-side unshard is a pure concat/interleave.

v2 changes vs baseline:
  - host pre-transposes + pre-casts x^T, qkv_w^T, out_w^T to bf16
    (removes all gpsimd casts and PE transposes of x/weights)
  - exp split across Scalar (exact), DVE + Pool (Schraudolph bit-exp:
    int16(S*A+B) bit-cast as bf16)
  - software-pipelined emission: qkv(b1) interleaved into attention(b0),
    out-proj(b0) into attention(b1); AllToAll per batch
  - normalization via partition-broadcast + single wide divide
"""

import numpy as np

HIDDEN = 1024
HEAD_DIM = 64
N_CORES = 8
B = 2

# attention mb-pair schedule: 16 slots of 2 key-blocks each.
# "act" slots: exact exp on ScalarE -> fp8 P -> DoubleRow PV (2 blocks/matmul)
# "dve" slots: bit-exp on DVE -> bf16 P -> 2 plain PV matmuls
ACT_PAIRS = 12
DVE_PAIRS = 4

# Schraudolph bf16 bit-exp: int16(round(s * A + B)) bitcast to bf16
# approximates exp(s/8).  A = 184.6650 * 0.125; B calibrated on host
# (includes +0.5 truncation compensation).
BITEXP_A = 184.6650 / 8.0
BITEXP_B = 16249.0


def _exp_schedule():
    """Interleaved engine assignment for the 16 mb-pair slots of one qc."""
    counts = {"act": ACT_PAIRS, "dve": DVE_PAIRS}
    counts = {k: v for k, v in counts.items() if v > 0}
    total = sum(counts.values())
    assert total == 16
    sched = []
    acc = {k: 0.0 for k in counts}
    for _ in range(16):
        for k in counts:
            acc[k] += counts[k] / total
        pick = max(acc, key=lambda k: acc[k])
        acc[pick] -= 1.0
        sched.append(pick)
    return sched


def build_nc(n_tok_b=4096, n_cores=8, hidden=1024):
    import concourse.bass as bass
    import concourse.bacc as bacc
    import concourse.tile as tile
    import concourse.mybir as mybir
    from concourse.masks import make_identity

    f32 = mybir.dt.float32
    bf16 = mybir.dt.bfloat16
    i16 = mybir.dt.int16
    f8 = mybir.dt.float8e4
    MPM = mybir.MatmulPerfMode
    AF = mybir.ActivationFunctionType
    ALU = mybir.AluOpType

    C = hidden
    CB = C // 128             # 8 contraction blocks
    assert CB == n_cores
    NB = n_tok_b              # tokens per batch
    T = B * NB
    QC = 512                  # query chunk
    NQC = NB // QC            # 8
    NMB = NB // 128           # 32 key blocks per batch
    GRP = 512                 # qkv token group
    NGRP = NB // GRP          # 8  (== NQC, used for interleaving)
    TSL_B = NB // n_cores     # 512-token out-proj slice per core per batch

    sched = _exp_schedule()

    nc = bacc.Bacc("TRN2", target_bir_lowering=False, debug=False,
                   num_devices=n_cores)

    xT_d = nc.declare_dram_parameter("xT", [CB, 128, T], bf16, isOutput=False)
    wT_d = nc.declare_dram_parameter("wT", [3, 128, CB * 128], bf16,
                                     isOutput=False)
    qkvb_d = nc.declare_dram_parameter("qkvb", [3, 128, 1], f32,
                                       isOutput=False)
    owT_d = nc.declare_dram_parameter("owT", [CB, 128, C], bf16,
                                      isOutput=False)
    outb_d = nc.declare_dram_parameter("outb", [1, C], f32, isOutput=False)
    out_d = nc.declare_dram_parameter("out", [B * TSL_B, C], f32,
                                      isOutput=True)

    a2a_in = [nc.dram_tensor(f"a2a_in{b}", [n_cores, 128, TSL_B], bf16)
              for b in range(B)]
    a2a_out = [nc.dram_tensor(f"a2a_out{b}", [n_cores, 128, TSL_B], bf16)
               for b in range(B)]

    with tile.TileContext(nc) as tc:
        with (
            tc.tile_pool(name="persist", bufs=1) as pp,
            tc.tile_pool(name="xt", bufs=2) as xtp,
            tc.tile_pool(name="pexp", bufs=6) as pexpp,
            tc.tile_pool(name="osbp", bufs=6) as osbp,
            tc.tile_pool(name="misc", bufs=2) as mp,
            tc.tile_pool(name="stp", bufs=3, space="PSUM") as stp,
            tc.tile_pool(name="ohp", bufs=2, space="PSUM") as ohp,
        ):
            ident = pp.tile([128, 128], bf16, tag="ident")
            make_identity(nc, ident)

            # ---- persistent weights ----
            wT = pp.tile([128, 3 * CB * 128], bf16, tag="wT")
            wT4 = wT[:].rearrange("p (m cb d) -> p m cb d", m=3, cb=CB)
            for m in range(3):
                nc.sync.dma_start(
                    wT[:, m * CB * 128:(m + 1) * CB * 128], wT_d[m])
            owT = pp.tile([128, CB * C], bf16, tag="owT")
            owT3 = owT[:].rearrange("p (g co) -> p g co", co=C)
            for g in range(CB):
                nc.sync.dma_start(owT3[:, g], owT_d[g])
            bias_sb = pp.tile([128, 3], f32, tag="bias")
            for m in range(3):
                nc.sync.dma_start(bias_sb[:, m:m + 1], qkvb_d[m])
            outb_f = pp.tile([1, C], f32, tag="outbf")
            nc.sync.dma_start(outb_f[:], outb_d[:])
            outb_sb = pp.tile([1, C], bf16, tag="outb")
            nc.vector.tensor_copy(outb_sb[:], outb_f[:])
            ones_sb = pp.tile([1, 128], bf16, tag="ones")
            nc.vector.memset(ones_sb[:], 1.0)

            # ---- per-batch persistent tensors ----
            qT = [pp.tile([128, NB], bf16, tag=f"qT{b}", name=f"qT{b}")
                  for b in range(B)]
            kT = [pp.tile([128, NB], bf16, tag=f"kT{b}", name=f"kT{b}")
                  for b in range(B)]
            V = [pp.tile([128, NMB * 130], bf16, tag=f"V{b}", name=f"V{b}")
                 for b in range(B)]
            V8 = [pp.tile([128, NMB * 144], f8, tag=f"V8{b}", name=f"V8{b}")
                  for b in range(B)]
            for b in range(B):
                nc.vector.memset(V[b][:], 1.0)
            Oh0 = pp.tile([64, T], bf16, tag="Oh0")
            Oh1 = pp.tile([64, T], bf16, tag="Oh1")
            recv = [pp.tile([128, n_cores * TSL_B], bf16, tag=f"recv{b}",
                             name=f"recv{b}") for b in range(B)]
            # normalization scratch: per (qc mod 4, head) slot
            rcb = pp.tile([1, 4 * QC], bf16, tag="rcb")
            rc = pp.tile([1, 4 * QC], f32, tag="rc")
            rcp = pp.tile([1, 4 * QC], f32, tag="rcp")
            rb = pp.tile([128, 4 * QC], f32, tag="rb")

            def qkv_group(b, grp):
                """qkv projection for 512 tokens of batch b (generator:
                yields between chunks so attention emission can interleave
                finely and keep the exp engines fed)."""
                t0 = b * NB + grp * GRP
                xt = xtp.tile([128, CB * GRP], bf16, tag="xt")
                xt3 = xt[:].rearrange("p (cb t) -> p cb t", t=GRP)
                for cb in range(CB):
                    nc.sync.dma_start(xt3[:, cb], xT_d[cb, :, t0:t0 + GRP])
                yield
                for m in range(3):
                    qp = stp.tile([128, 2 * GRP], f32, tag="st")
                    for cb in range(CB):
                        nc.tensor.matmul(qp[:, 0:GRP], wT4[:, m, cb],
                                         xt3[:, cb],
                                         start=(cb == 0), stop=(cb == CB - 1))
                        if cb == 3:
                            yield
                    if m < 2:
                        dest = (qT if m == 0 else kT)[b][
                            :, grp * GRP:(grp + 1) * GRP]
                        nc.vector.tensor_scalar(dest, qp[:, 0:GRP],
                                                bias_sb[:, m:m + 1],
                                                None, op0=ALU.add)
                        yield
                    else:
                        vs = mp.tile([128, GRP], bf16, tag="vs")
                        nc.vector.tensor_scalar(vs[:], qp[:, 0:GRP],
                                                bias_sb[:, 2:3],
                                                None, op0=ALU.add)
                        tp = stp.tile([128, 2 * GRP], bf16, tag="st")
                        for j in range(GRP // 128):
                            nc.tensor.transpose(
                                tp[:, j * 128:(j + 1) * 128],
                                vs[:, j * 128:(j + 1) * 128], ident[:])
                        mb0 = grp * (GRP // 128)
                        vv = V[b][:].rearrange("p (m d) -> p m d", d=130)
                        tp3 = tp[:, 0:GRP].rearrange("p (j a) -> p j a", a=128)
                        nc.vector.tensor_copy(vv[:, mb0:mb0 + 4, 0:64],
                                              tp3[:, :, 0:64])
                        nc.vector.tensor_copy(vv[:, mb0:mb0 + 4, 65:129],
                                              tp3[:, :, 64:128])
                        v84 = V8[b][:].rearrange("p (m d) -> p m d", d=144)
                        nc.gpsimd.tensor_copy(
                            v84[:, mb0:mb0 + 4, 0:130],
                            vv[:, mb0:mb0 + 4, 0:130])
                        yield

            def attention_qc(b, qc, fins_out, filler=None):
                """S + exp + PV for one 512-query chunk (generator: yields
                after each of the 16 slots).  Deferred normalize closures
                are appended to fins_out.  `filler` is an optional
                generator stepped at every slot."""
                qsl = slice(qc * QC, (qc + 1) * QC)
                oh0 = ohp.tile([65, QC], f32, tag="oh")
                oh1 = ohp.tile([65, QC], f32, tag="oh")
                V83 = V8[b][:].rearrange("p (m d) -> p m d", d=144)

                def s_one(mb):
                    st = stp.tile([128, 2 * QC], f32, tag="st")
                    nc.tensor.matmul(st[:, 0:QC],
                                     kT[b][0:64, mb * 128:mb * 128 + 128],
                                     qT[b][0:64, qsl],
                                     start=True, stop=True)
                    nc.tensor.matmul(st[:, QC:2 * QC],
                                     kT[b][64:128, mb * 128:mb * 128 + 128],
                                     qT[b][64:128, qsl],
                                     start=True, stop=True)
                    return st

                def pv_dr(slot, pe8):
                    """DoubleRow PV covering key blocks 2*slot, 2*slot+1."""
                    pe83 = pe8[:].rearrange("p (m q) -> p m q", q=2 * QC)
                    first = (slot == 0)
                    last = (slot == 15)
                    nc.tensor.matmul(
                        oh0[:], V83[:, 2 * slot:2 * slot + 2, 0:65],
                        pe83[:, :, 0:QC],
                        start=first, stop=last, perf_mode=MPM.DoubleRow)
                    nc.tensor.matmul(
                        oh1[:], V83[:, 2 * slot:2 * slot + 2, 65:130],
                        pe83[:, :, QC:2 * QC],
                        start=first, stop=last, perf_mode=MPM.DoubleRow)

                def pv_bf(mb, pe, slot, half):
                    first = (slot == 0 and half == 0)
                    last = (slot == 15 and half == 1)
                    nc.tensor.matmul(oh0[:],
                                     V[b][:, mb * 130:mb * 130 + 65],
                                     pe[:, 0:QC],
                                     start=first, stop=last)
                    nc.tensor.matmul(oh1[:],
                                     V[b][:, mb * 130 + 65:mb * 130 + 130],
                                     pe[:, QC:2 * QC],
                                     start=first, stop=last)

                def step_filler():
                    if filler is not None:
                        try:
                            next(filler)
                        except StopIteration:
                            pass

                sts = [s_one(0), s_one(1)]
                for slot in range(16):
                    st0, st1 = sts
                    if slot + 1 < 16:
                        sts = [s_one(2 * slot + 2), s_one(2 * slot + 3)]
                    step_filler()
                    if sched[slot] == "act":
                        pe8 = pexpp.tile([128, 4 * QC], f8, tag="pe")
                        nc.scalar.activation(pe8[:, 0:2 * QC], st0[:],
                                             AF.Exp, scale=0.125)
                        nc.scalar.activation(pe8[:, 2 * QC:4 * QC], st1[:],
                                             AF.Exp, scale=0.125)
                        pv_dr(slot, pe8)
                    else:
                        pe0 = pexpp.tile([128, 2 * QC], bf16, tag="pe")
                        pe1 = pexpp.tile([128, 2 * QC], bf16, tag="pe")
                        nc.vector.tensor_scalar(pe0[:].bitcast(i16), st0[:],
                                                BITEXP_A, BITEXP_B,
                                                op0=ALU.mult, op1=ALU.add)
                        nc.vector.tensor_scalar(pe1[:].bitcast(i16), st1[:],
                                                BITEXP_A, BITEXP_B,
                                                op0=ALU.mult, op1=ALU.add)
                        pv_bf(2 * slot, pe0, slot, 0)
                        pv_bf(2 * slot + 1, pe1, slot, 1)
                    yield
                if filler is not None:
                    for _ in filler:
                        pass
                # stage O+den to SBUF immediately (frees the oh PSUM
                # banks); the gpsimd-dependent normalize is returned as a
                # deferred closure so a collective occupying the gpsimd
                # queue cannot stall this qc's PSUM rotation.
                for h, oh in ((0, oh0), (1, oh1)):
                    sl = slice(((qc % 2) * 2 + h) * QC,
                               ((qc % 2) * 2 + h + 1) * QC)
                    osb = osbp.tile([65, QC], bf16, tag="osb")
                    nc.vector.tensor_copy(osb[:], oh[:])
                    nc.sync.dma_start(rcb[0:1, sl], osb[64:65, :])
                    dest = (Oh0 if h == 0 else Oh1)[
                        :, b * NB + qc * QC: b * NB + (qc + 1) * QC]

                    def fin(sl=sl, osb=osb, dest=dest):
                        nc.vector.tensor_copy(rc[0:1, sl], rcb[0:1, sl])
                        nc.vector.reciprocal_approx_fast(rcp[0:1, sl],
                                                         rc[0:1, sl])
                        nc.gpsimd.partition_broadcast(rb[0:64, sl],
                                                      rcp[0:1, sl])
                        nc.vector.scalar_tensor_tensor(
                            dest, osb[0:64, :], 1.0, rb[0:64, sl],
                            op0=ALU.mult, op1=ALU.mult)
                    fins_out.append(fin)

            def a2a_launch(b):
                bsl = slice(b * NB, (b + 1) * NB)
                for j in range(n_cores):
                    nc.sync.dma_start(
                        a2a_in[b][j, 0:64, :],
                        Oh0[:, b * NB + j * TSL_B: b * NB + (j + 1) * TSL_B])
                    nc.sync.dma_start(
                        a2a_in[b][j, 64:128, :],
                        Oh1[:, b * NB + j * TSL_B: b * NB + (j + 1) * TSL_B])
                nc.gpsimd.collective_compute(
                    "AllToAll", ALU.bypass,
                    replica_groups=[list(range(n_cores))],
                    ins=[a2a_in[b].ap().opt()],
                    outs=[a2a_out[b].ap().opt()],
                )
                for g in range(n_cores):
                    nc.sync.dma_start(
                        recv[b][:, g * TSL_B:(g + 1) * TSL_B], a2a_out[b][g])

            def outproj_tb(b, tb):
                """out projection for 128 tokens of my slice of batch b."""
                recv3 = recv[b][:].rearrange("p (g t) -> p g t", t=TSL_B)
                ot = mp.tile([128, C], f32, tag="ot")
                for co2 in range(C // 512):
                    pj = stp.tile([128, 2 * QC], f32, tag="st")
                    for g in range(n_cores):
                        nc.tensor.matmul(
                            pj[:, 0:512],
                            recv3[:, g, tb * 128:tb * 128 + 128],
                            owT3[:, g, co2 * 512:(co2 + 1) * 512],
                            start=(g == 0), stop=False)
                    nc.tensor.matmul(pj[:, 0:512], ones_sb[:],
                                     outb_sb[:, co2 * 512:(co2 + 1) * 512],
                                     start=False, stop=True)
                    nc.vector.tensor_copy(ot[:, co2 * 512:(co2 + 1) * 512],
                                          pj[:, 0:512])
                nc.sync.dma_start(
                    out_d[b * TSL_B + tb * 128: b * TSL_B + (tb + 1) * 128, :],
                    ot[:])

            # ================= pipeline =================
            from itertools import chain

            def drain(g):
                for _ in g:
                    pass

            pending = []

            def flush_pending(n=None):
                k = len(pending) if n is None else n
                for _ in range(k):
                    if pending:
                        pending.pop(0)()

            # qkv(b0) overlapped with attention(b0, qc0): slot s only needs
            # kT/V groups <= (2s+1)//4, so groups 2..7 stream while qc0's
            # early slots already run (keeps Act/DVE fed from the start).
            g0 = [qkv_group(0, g) for g in range(NGRP)]
            drain(g0[0])
            drain(g0[1])
            att = attention_qc(0, 0, pending)
            done = 0
            for g in range(2, NGRP):
                alive = True
                tick = 0
                while alive:
                    try:
                        next(g0[g])
                    except StopIteration:
                        alive = False
                    tick += 1
                    if tick % 2 == 0 and done < min(2 * g - 1, 16):
                        next(att)
                        done += 1
            drain(att)
            flush_pending(2)
            for qc in range(1, NQC):
                filler = (chain(qkv_group(1, qc - 1), qkv_group(1, NGRP - 1))
                          if qc == NQC - 1 else qkv_group(1, qc - 1))
                drain(attention_qc(0, qc, pending, filler=filler))
                flush_pending(2)
            flush_pending()
            a2a_launch(0)
            for qc in range(NQC):
                drain(attention_qc(1, qc, pending))
                if qc >= 1:
                    flush_pending(2)
                if qc % 2 == 1:
                    outproj_tb(0, qc // 2)
            flush_pending()
            a2a_launch(1)
            for tb in range(TSL_B // 128):
                outproj_tb(1, tb)

    nc.compile()
    return nc


def shard_inputs(x, qkv_w, qkv_b, out_w, out_b, n_cores=8):
    """Per-core input maps with host-side transpose + bf16 cast."""
    import ml_dtypes
    bf = ml_dtypes.bfloat16
    Bv, N, C = x.shape
    T = Bv * N
    CB = C // 128
    # xT [CB, 128, T]
    xT = np.ascontiguousarray(
        x.reshape(T, CB, 128).transpose(1, 2, 0).astype(bf))
    # owT [CB, 128, C]: owT[cb, p, co] = out_w[co, cb*128+p]
    owT = np.ascontiguousarray(
        out_w.astype(bf).T.reshape(CB, 128, C))
    outb = np.ascontiguousarray(out_b.reshape(1, C).astype(np.float32))
    in_maps = []
    for c in range(n_cores):
        r0 = c * 128
        # wT [3, 128, CB*128]: wT[m, p, cb*128+d] = qkv_w[m*C+r0+d, cb*128+p]
        w = np.stack([qkv_w[m * C + r0: m * C + r0 + 128] for m in range(3)])
        wT = np.ascontiguousarray(
            w.astype(bf).reshape(3, 128, CB, 128)
            .transpose(0, 3, 2, 1).reshape(3, 128, CB * 128))
        bvec = np.stack([qkv_b[m * C + r0: m * C + r0 + 128]
                         for m in range(3)])[:, :, None]
        in_maps.append({
            "xT": xT,
            "wT": wT,
            "qkvb": np.ascontiguousarray(bvec.astype(np.float32)),
            "owT": owT,
            "outb": outb,
        })
    return in_maps


def unshard(results, Bv, N, C, n_cores=8):
    """results[c]["out"] is [B*TSL_B, C]: batch-major 512-token slices."""
    TSL_B = N // n_cores
    out = np.empty((Bv, N, C), dtype=np.float32)
    for c in range(n_cores):
        o = results[c]["out"]
        for b in range(Bv):
            out[b, c * TSL_B:(c + 1) * TSL_B, :] = \
                o[b * TSL_B:(b + 1) * TSL_B]
    return out


_NC_CACHE = {}


def kernel(x, qkv_w, qkv_b, out_w, out_b):
    from concourse import bass_utils
    x = np.asarray(x)
    Bv, N, C = x.shape
    key = (N, C)
    if key not in _NC_CACHE:
        _NC_CACHE[key] = build_nc(n_tok_b=N, n_cores=N_CORES, hidden=C)
    nc = _NC_CACHE[key]
    in_maps = shard_inputs(x, np.asarray(qkv_w), np.asarray(qkv_b),
                           np.asarray(out_w), np.asarray(out_b),
                           n_cores=N_CORES)
    res = bass_utils.run_bass_kernel_spmd(nc, in_maps,
                                          core_ids=list(range(N_CORES)))
    return unshard(res.results, Bv, N, C, n_cores=N_CORES)


# revision 4
# speedup vs baseline: 1.0996x; 1.0996x over previous
"""Distributed AttentionBlock kernel for 8 TRN2 NeuronCores.

Sharding: tensor-parallel over heads (16 heads -> 2 per core) for
qkv-projection + attention; per-batch AllToAll redistributes attention
output so each core computes the out-projection for a 512-token slice of
each batch; host-side unshard is a pure concat/interleave.

v2 changes vs baseline:
  - host pre-transposes + pre-casts x^T, qkv_w^T, out_w^T to bf16
    (removes all gpsimd casts and PE transposes of x/weights)
  - exp split across Scalar (exact), DVE + Pool (Schraudolph bit-exp:
    int16(S*A+B) bit-cast as bf16)
  - software-pipelined emission: qkv(b1) interleaved into attention(b0),
    out-proj(b0) into attention(b1); AllToAll per batch
  - normalization via partition-broadcast + single wide divide
"""

import numpy as np

HIDDEN = 1024
HEAD_DIM = 64
N_CORES = 8
B = 2

# attention mb-pair schedule: 16 slots of 2 key-blocks each.
# "act" slots: exact exp on ScalarE -> fp8 P -> DoubleRow PV (2 blocks/matmul)
# "dve" slots: bit-exp on DVE -> bf16 P -> 2 plain PV matmuls
ACT_PAIRS = 12
DVE_PAIRS = 4

# Schraudolph bf16 bit-exp: int16(round(s * A + B)) bitcast to bf16
# approximates exp(s/8).  A = 184.6650 * 0.125; B calibrated on host
# (includes +0.5 truncation compensation).
BITEXP_A = 184.6650 / 8.0
BITEXP_B = 16249.0


def _exp_schedule():
    """Interleaved engine assignment for the 16 mb-pair slots of one qc."""
    counts = {"act": ACT_PAIRS, "dve": DVE_PAIRS}
    counts = {k: v for k, v in counts.items() if v > 0}
    total = sum(counts.values())
    assert total == 16
    sched = []
    acc = {k: 0.0 for k in counts}
    for _ in range(16):
        for k in counts:
            acc[k] += counts[k] / total
        pick = max(acc, key=lambda k: acc[k])
        acc[pick] -= 1.0
        sched.append(pick)
    return sched


def build_nc(n_tok_b=4096, n_cores=8, hidden=1024):
    import concourse.bass as bass
    import concourse.bacc as bacc
    import concourse.tile as tile
    import concourse.mybir as mybir
    from concourse.masks import make_identity

    f32 = mybir.dt.float32
    bf16 = mybir.dt.bfloat16
    i16 = mybir.dt.int16
    f8 = mybir.dt.float8e4
    MPM = mybir.MatmulPerfMode
    AF = mybir.ActivationFunctionType
    ALU = mybir.AluOpType

    C = hidden
    CB = C // 128             # 8 contraction blocks
    assert CB == n_cores
    NB = n_tok_b              # tokens per batch
    T = B * NB
    QC = 512                  # query chunk
    NQC = NB // QC            # 8
    NMB = NB // 128           # 32 key blocks per batch
    GRP = 512                 # qkv token group
    NGRP = NB // GRP          # 8  (== NQC, used for interleaving)
    TSL_B = NB // n_cores     # 512-token out-proj slice per core per batch

    sched = _exp_schedule()

    nc = bacc.Bacc("TRN2", target_bir_lowering=False, debug=False,
                   num_devices=n_cores)

    xT_d = nc.declare_dram_parameter("xT", [CB, 128, T], bf16, isOutput=False)
    wT_d = nc.declare_dram_parameter("wT", [3, 128, CB * 128], bf16,
                                     isOutput=False)
    qkvb_d = nc.declare_dram_parameter("qkvb", [3, 128, 1], f32,
                                       isOutput=False)
    owT_d = nc.declare_dram_parameter("owT", [CB, 128, C], bf16,
                                      isOutput=False)
    outb_d = nc.declare_dram_parameter("outb", [1, C], f32, isOutput=False)
    out_d = nc.declare_dram_parameter("out", [B * TSL_B, C], f32,
                                      isOutput=True)

    a2a_in = [nc.dram_tensor(f"a2a_in{b}", [n_cores, 128, TSL_B], bf16)
              for b in range(B)]
    a2a_out = [nc.dram_tensor(f"a2a_out{b}", [n_cores, 128, TSL_B], bf16)
               for b in range(B)]

    with tile.TileContext(nc) as tc:
        with (
            tc.tile_pool(name="persist", bufs=1) as pp,
            tc.tile_pool(name="xt", bufs=2) as xtp,
            tc.tile_pool(name="pexp", bufs=6) as pexpp,
            tc.tile_pool(name="osbp", bufs=6) as osbp,
            tc.tile_pool(name="misc", bufs=2) as mp,
            tc.tile_pool(name="stp", bufs=3, space="PSUM") as stp,
            tc.tile_pool(name="ohp", bufs=2, space="PSUM") as ohp,
        ):
            ident = pp.tile([128, 128], bf16, tag="ident")
            make_identity(nc, ident)

            # ---- persistent weights ----
            wT = pp.tile([128, 3 * CB * 128], bf16, tag="wT")
            wT4 = wT[:].rearrange("p (m cb d) -> p m cb d", m=3, cb=CB)
            for m in range(3):
                nc.sync.dma_start(
                    wT[:, m * CB * 128:(m + 1) * CB * 128], wT_d[m])
            owT = pp.tile([128, CB * C], bf16, tag="owT")
            owT3 = owT[:].rearrange("p (g co) -> p g co", co=C)
            for g in range(CB):
                nc.sync.dma_start(owT3[:, g], owT_d[g])
            bias_sb = pp.tile([128, 3], f32, tag="bias")
            for m in range(3):
                nc.sync.dma_start(bias_sb[:, m:m + 1], qkvb_d[m])
            outb_f = pp.tile([1, C], f32, tag="outbf")
            nc.sync.dma_start(outb_f[:], outb_d[:])
            outb_sb = pp.tile([1, C], bf16, tag="outb")
            nc.vector.tensor_copy(outb_sb[:], outb_f[:])
            ones_sb = pp.tile([1, 128], bf16, tag="ones")
            nc.vector.memset(ones_sb[:], 1.0)

            # ---- per-batch persistent tensors ----
            qT = [pp.tile([128, NB], bf16, tag=f"qT{b}", name=f"qT{b}")
                  for b in range(B)]
            kT = [pp.tile([128, NB], bf16, tag=f"kT{b}", name=f"kT{b}")
                  for b in range(B)]
            V = [pp.tile([128, NMB * 130], bf16, tag=f"V{b}", name=f"V{b}")
                 for b in range(B)]
            V8 = [pp.tile([128, NMB * 144], f8, tag=f"V8{b}", name=f"V8{b}")
                  for b in range(B)]
            for b in range(B):
                nc.vector.memset(V[b][:], 1.0)
            Oh0 = pp.tile([64, T], bf16, tag="Oh0")
            Oh1 = pp.tile([64, T], bf16, tag="Oh1")
            recv = [pp.tile([128, n_cores * TSL_B], bf16, tag=f"recv{b}",
                             name=f"recv{b}") for b in range(B)]
            # normalization scratch: per (qc mod 4, head) slot
            rcb = pp.tile([1, 4 * QC], bf16, tag="rcb")
            rc = pp.tile([1, 4 * QC], f32, tag="rc")
            rcp = pp.tile([1, 4 * QC], f32, tag="rcp")
            rb = pp.tile([128, 4 * QC], f32, tag="rb")

            def qkv_group(b, grp):
                """qkv projection for 512 tokens of batch b (generator:
                yields between chunks so attention emission can interleave
                finely and keep the exp engines fed)."""
                t0 = b * NB + grp * GRP
                xt = xtp.tile([128, CB * GRP], bf16, tag="xt")
                xt3 = xt[:].rearrange("p (cb t) -> p cb t", t=GRP)
                for cb in range(CB):
                    nc.sync.dma_start(xt3[:, cb], xT_d[cb, :, t0:t0 + GRP])
                yield
                for m in range(3):
                    qp = stp.tile([128, 2 * GRP], f32, tag="st")
                    for cb in range(CB):
                        nc.tensor.matmul(qp[:, 0:GRP], wT4[:, m, cb],
                                         xt3[:, cb],
                                         start=(cb == 0), stop=(cb == CB - 1))
                        if cb == 3:
                            yield
                    if m < 2:
                        dest = (qT if m == 0 else kT)[b][
                            :, grp * GRP:(grp + 1) * GRP]
                        nc.vector.tensor_scalar(dest, qp[:, 0:GRP],
                                                bias_sb[:, m:m + 1],
                                                None, op0=ALU.add)
                        yield
                    else:
                        vs = mp.tile([128, GRP], bf16, tag="vs")
                        nc.vector.tensor_scalar(vs[:], qp[:, 0:GRP],
                                                bias_sb[:, 2:3],
                                                None, op0=ALU.add)
                        tp = stp.tile([128, 2 * GRP], bf16, tag="st")
                        for j in range(GRP // 128):
                            nc.tensor.transpose(
                                tp[:, j * 128:(j + 1) * 128],
                                vs[:, j * 128:(j + 1) * 128], ident[:])
                        mb0 = grp * (GRP // 128)
                        vv = V[b][:].rearrange("p (m d) -> p m d", d=130)
                        tp3 = tp[:, 0:GRP].rearrange("p (j a) -> p j a", a=128)
                        nc.vector.tensor_copy(vv[:, mb0:mb0 + 4, 0:64],
                                              tp3[:, :, 0:64])
                        nc.vector.tensor_copy(vv[:, mb0:mb0 + 4, 65:129],
                                              tp3[:, :, 64:128])
                        v84 = V8[b][:].rearrange("p (m d) -> p m d", d=144)
                        nc.gpsimd.tensor_copy(
                            v84[:, mb0:mb0 + 4, 0:130],
                            vv[:, mb0:mb0 + 4, 0:130])
                        yield

            def attention_qc(b, qc, fins_out, filler=None):
                """S + exp + PV for one 512-query chunk (generator: yields
                after each of the 16 slots).  Deferred normalize closures
                are appended to fins_out.  `filler` is an optional
                generator stepped at every slot."""
                qsl = slice(qc * QC, (qc + 1) * QC)
                oh0 = ohp.tile([65, QC], f32, tag="oh")
                oh1 = ohp.tile([65, QC], f32, tag="oh")
                V83 = V8[b][:].rearrange("p (m d) -> p m d", d=144)

                def s_one(mb):
                    st = stp.tile([128, 2 * QC], f32, tag="st")
                    nc.tensor.matmul(st[:, 0:QC],
                                     kT[b][0:64, mb * 128:mb * 128 + 128],
                                     qT[b][0:64, qsl],
                                     start=True, stop=True)
                    nc.tensor.matmul(st[:, QC:2 * QC],
                                     kT[b][64:128, mb * 128:mb * 128 + 128],
                                     qT[b][64:128, qsl],
                                     start=True, stop=True)
                    return st

                def pv_dr(slot, pe8):
                    """DoubleRow PV covering key blocks 2*slot, 2*slot+1."""
                    pe83 = pe8[:].rearrange("p (m q) -> p m q", q=2 * QC)
                    first = (slot == 0)
                    last = (slot == 15)
                    nc.tensor.matmul(
                        oh0[:], V83[:, 2 * slot:2 * slot + 2, 0:65],
                        pe83[:, :, 0:QC],
                        start=first, stop=last, perf_mode=MPM.DoubleRow)
                    nc.tensor.matmul(
                        oh1[:], V83[:, 2 * slot:2 * slot + 2, 65:130],
                        pe83[:, :, QC:2 * QC],
                        start=first, stop=last, perf_mode=MPM.DoubleRow)

                def pv_bf(mb, pe, slot, half):
                    first = (slot == 0 and half == 0)
                    last = (slot == 15 and half == 1)
                    nc.tensor.matmul(oh0[:],
                                     V[b][:, mb * 130:mb * 130 + 65],
                                     pe[:, 0:QC],
                                     start=first, stop=last)
                    nc.tensor.matmul(oh1[:],
                                     V[b][:, mb * 130 + 65:mb * 130 + 130],
                                     pe[:, QC:2 * QC],
                                     start=first, stop=last)

                def step_filler():
                    if filler is not None:
                        try:
                            next(filler)
                        except StopIteration:
                            pass

                sts = [s_one(0), s_one(1)]
                for slot in range(16):
                    st0, st1 = sts
                    if slot + 1 < 16:
                        sts = [s_one(2 * slot + 2), s_one(2 * slot + 3)]
                    step_filler()
                    if sched[slot] == "act":
                        pe8 = pexpp.tile([128, 4 * QC], f8, tag="pe")
                        nc.scalar.activation(pe8[:, 0:2 * QC], st0[:],
                                             AF.Exp, scale=0.125)
                        nc.scalar.activation(pe8[:, 2 * QC:4 * QC], st1[:],
                                             AF.Exp, scale=0.125)
                        pv_dr(slot, pe8)
                    else:
                        pe0 = pexpp.tile([128, 2 * QC], bf16, tag="pe")
                        pe1 = pexpp.tile([128, 2 * QC], bf16, tag="pe")
                        nc.vector.tensor_scalar(pe0[:].bitcast(i16), st0[:],
                                                BITEXP_A, BITEXP_B,
                                                op0=ALU.mult, op1=ALU.add)
                        nc.vector.tensor_scalar(pe1[:].bitcast(i16), st1[:],
                                                BITEXP_A, BITEXP_B,
                                                op0=ALU.mult, op1=ALU.add)
                        pv_bf(2 * slot, pe0, slot, 0)
                        pv_bf(2 * slot + 1, pe1, slot, 1)
                    yield
                if filler is not None:
                    for _ in filler:
                        pass
                # stage O+den to SBUF immediately (frees the oh PSUM
                # banks); the gpsimd-dependent normalize is returned as a
                # deferred closure so a collective occupying the gpsimd
                # queue cannot stall this qc's PSUM rotation.
                for h, oh in ((0, oh0), (1, oh1)):
                    sl = slice(((qc % 2) * 2 + h) * QC,
                               ((qc % 2) * 2 + h + 1) * QC)
                    osb = osbp.tile([65, QC], bf16, tag="osb")
                    nc.vector.tensor_copy(osb[:], oh[:])
                    nc.sync.dma_start(rcb[0:1, sl], osb[64:65, :])
                    dest = (Oh0 if h == 0 else Oh1)[
                        :, b * NB + qc * QC: b * NB + (qc + 1) * QC]

                    def fin(sl=sl, osb=osb, dest=dest):
                        nc.vector.tensor_copy(rc[0:1, sl], rcb[0:1, sl])
                        nc.vector.reciprocal_approx_fast(rcp[0:1, sl],
                                                         rc[0:1, sl])
                        nc.gpsimd.partition_broadcast(rb[0:64, sl],
                                                      rcp[0:1, sl])
                        nc.vector.scalar_tensor_tensor(
                            dest, osb[0:64, :], 1.0, rb[0:64, sl],
                            op0=ALU.mult, op1=ALU.mult)
                    fins_out.append(fin)

            def a2a_launch(b):
                bsl = slice(b * NB, (b + 1) * NB)
                for j in range(n_cores):
                    nc.sync.dma_start(
                        a2a_in[b][j, 0:64, :],
                        Oh0[:, b * NB + j * TSL_B: b * NB + (j + 1) * TSL_B])
                    nc.sync.dma_start(
                        a2a_in[b][j, 64:128, :],
                        Oh1[:, b * NB + j * TSL_B: b * NB + (j + 1) * TSL_B])
                nc.gpsimd.collective_compute(
                    "AllToAll", ALU.bypass,
                    replica_groups=[list(range(n_cores))],
                    ins=[a2a_in[b].ap().opt()],
                    outs=[a2a_out[b].ap().opt()],
                )
                for g in range(n_cores):
                    nc.sync.dma_start(
                        recv[b][:, g * TSL_B:(g + 1) * TSL_B], a2a_out[b][g])

            def outproj_tb(b, tb):
                """out projection for 128 tokens of my slice of batch b."""
                recv3 = recv[b][:].rearrange("p (g t) -> p g t", t=TSL_B)
                ot = mp.tile([128, C], f32, tag="ot")
                for co2 in range(C // 512):
                    pj = stp.tile([128, 2 * QC], f32, tag="st")
                    for g in range(n_cores):
                        nc.tensor.matmul(
                            pj[:, 0:512],
                            recv3[:, g, tb * 128:tb * 128 + 128],
                            owT3[:, g, co2 * 512:(co2 + 1) * 512],
                            start=(g == 0), stop=False)
                    nc.tensor.matmul(pj[:, 0:512], ones_sb[:],
                                     outb_sb[:, co2 * 512:(co2 + 1) * 512],
                                     start=False, stop=True)
                    nc.vector.tensor_copy(ot[:, co2 * 512:(co2 + 1) * 512],
                                          pj[:, 0:512])
                nc.sync.dma_start(
                    out_d[b * TSL_B + tb * 128: b * TSL_B + (tb + 1) * 128, :],
                    ot[:])

            # ================= pipeline =================
            from itertools import chain

            def drain(g):
                for _ in g:
                    pass

            pending = []

            def flush_pending(n=None):
                k = len(pending) if n is None else n
                for _ in range(k):
                    if pending:
                        pending.pop(0)()

            # qkv(b0) overlapped with attention(b0, qc0): slot s only needs
            # kT/V groups <= (2s+1)//4, so groups 2..7 stream while qc0's
            # early slots already run (keeps Act/DVE fed from the start).
            g0 = [qkv_group(0, g) for g in range(NGRP)]
            drain(g0[0])
            drain(g0[1])
            att = attention_qc(0, 0, pending)
            done = 0
            for g in range(2, NGRP):
                alive = True
                tick = 0
                while alive:
                    try:
                        next(g0[g])
                    except StopIteration:
                        alive = False
                    tick += 1
                    if tick % 2 == 0 and done < min(2 * g - 1, 16):
                        next(att)
                        done += 1
            drain(att)
            flush_pending(2)
            for qc in range(1, NQC):
                filler = (chain(qkv_group(1, qc - 1), qkv_group(1, NGRP - 1))
                          if qc == NQC - 1 else qkv_group(1, qc - 1))
                drain(attention_qc(0, qc, pending, filler=filler))
                flush_pending(2)
            flush_pending()
            for qc in range(NQC):
                drain(attention_qc(1, qc, pending))
                if qc == 0:
                    a2a_launch(0)
                if qc >= 1:
                    flush_pending(2)
                if 2 <= qc <= 5:
                    outproj_tb(0, qc - 2)
            flush_pending()
            a2a_launch(1)
            for tb in range(TSL_B // 128):
                outproj_tb(1, tb)

    nc.compile()
    return nc


def shard_inputs(x, qkv_w, qkv_b, out_w, out_b, n_cores=8):
    """Per-core input maps with host-side transpose + bf16 cast."""
    import ml_dtypes
    bf = ml_dtypes.bfloat16
    Bv, N, C = x.shape
    T = Bv * N
    CB = C // 128
    # xT [CB, 128, T]
    xT = np.ascontiguousarray(
        x.reshape(T, CB, 128).transpose(1, 2, 0).astype(bf))
    # owT [CB, 128, C]: owT[cb, p, co] = out_w[co, cb*128+p]
    owT = np.ascontiguousarray(
        out_w.astype(bf).T.reshape(CB, 128, C))
    outb = np.ascontiguousarray(out_b.reshape(1, C).astype(np.float32))
    in_maps = []
    for c in range(n_cores):
        r0 = c * 128
        # wT [3, 128, CB*128]: wT[m, p, cb*128+d] = qkv_w[m*C+r0+d, cb*128+p]
        w = np.stack([qkv_w[m * C + r0: m * C + r0 + 128] for m in range(3)])
        wT = np.ascontiguousarray(
            w.astype(bf).reshape(3, 128, CB, 128)
            .transpose(0, 3, 2, 1).reshape(3, 128, CB * 128))
        bvec = np.stack([qkv_b[m * C + r0: m * C + r0 + 128]
                         for m in range(3)])[:, :, None]
        in_maps.append({
            "xT": xT,
            "wT": wT,
            "qkvb": np.ascontiguousarray(bvec.astype(np.float32)),
            "owT": owT,
            "outb": outb,
        })
    return in_maps


def unshard(results, Bv, N, C, n_cores=8):
    """results[c]["out"] is [B*TSL_B, C]: batch-major 512-token slices."""
    TSL_B = N // n_cores
    out = np.empty((Bv, N, C), dtype=np.float32)
    for c in range(n_cores):
        o = results[c]["out"]
        for b in range(Bv):
            out[b, c * TSL_B:(c + 1) * TSL_B, :] = \
                o[b * TSL_B:(b + 1) * TSL_B]
    return out


_NC_CACHE = {}


def kernel(x, qkv_w, qkv_b, out_w, out_b):
    from concourse import bass_utils
    x = np.asarray(x)
    Bv, N, C = x.shape
    key = (N, C)
    if key not in _NC_CACHE:
        _NC_CACHE[key] = build_nc(n_tok_b=N, n_cores=N_CORES, hidden=C)
    nc = _NC_CACHE[key]
    in_maps = shard_inputs(x, np.asarray(qkv_w), np.asarray(qkv_b),
                           np.asarray(out_w), np.asarray(out_b),
                           n_cores=N_CORES)
    res = bass_utils.run_bass_kernel_spmd(nc, in_maps,
                                          core_ids=list(range(N_CORES)))
    return unshard(res.results, Bv, N, C, n_cores=N_CORES)


# revision 5
# speedup vs baseline: 1.1435x; 1.0399x over previous
"""Distributed AttentionBlock kernel for 8 TRN2 NeuronCores.

Sharding: tensor-parallel over heads (16 heads -> 2 per core) for
qkv-projection + attention; per-batch AllToAll redistributes attention
output so each core computes the out-projection for a 512-token slice of
each batch; host-side unshard is a pure concat/interleave.

v2 changes vs baseline:
  - host pre-transposes + pre-casts x^T, qkv_w^T, out_w^T to bf16
    (removes all gpsimd casts and PE transposes of x/weights)
  - exp split across Scalar (exact), DVE + Pool (Schraudolph bit-exp:
    int16(S*A+B) bit-cast as bf16)
  - software-pipelined emission: qkv(b1) interleaved into attention(b0),
    out-proj(b0) into attention(b1); AllToAll per batch
  - normalization via partition-broadcast + single wide divide
"""

import numpy as np

HIDDEN = 1024
HEAD_DIM = 64
N_CORES = 8
B = 2

# attention mb-pair schedule: 16 slots of 2 key-blocks each.
# "act" slots: exact exp on ScalarE -> fp8 P -> DoubleRow PV (2 blocks/matmul)
# "dve" slots: bit-exp on DVE -> bf16 P -> 2 plain PV matmuls
ACT_PAIRS = 12
DVE_PAIRS = 4

# Schraudolph bf16 bit-exp: int16(round(s * A + B)) bitcast to bf16
# approximates exp(s/8).  A = 184.6650 * 0.125; B calibrated on host
# (includes +0.5 truncation compensation).
BITEXP_A = 184.6650 / 8.0
BITEXP_B = 16249.0


def _exp_schedule():
    """Interleaved engine assignment for the 16 mb-pair slots of one qc."""
    counts = {"act": ACT_PAIRS, "dve": DVE_PAIRS}
    counts = {k: v for k, v in counts.items() if v > 0}
    total = sum(counts.values())
    assert total == 16
    sched = []
    acc = {k: 0.0 for k in counts}
    for _ in range(16):
        for k in counts:
            acc[k] += counts[k] / total
        pick = max(acc, key=lambda k: acc[k])
        acc[pick] -= 1.0
        sched.append(pick)
    return sched


def build_nc(n_tok_b=4096, n_cores=8, hidden=1024):
    import concourse.bass as bass
    import concourse.bacc as bacc
    import concourse.tile as tile
    import concourse.mybir as mybir
    from concourse.masks import make_identity

    f32 = mybir.dt.float32
    bf16 = mybir.dt.bfloat16
    i16 = mybir.dt.int16
    f8 = mybir.dt.float8e4
    MPM = mybir.MatmulPerfMode
    AF = mybir.ActivationFunctionType
    ALU = mybir.AluOpType

    C = hidden
    CB = C // 128             # 8 contraction blocks
    assert CB == n_cores
    NB = n_tok_b              # tokens per batch
    T = B * NB
    QC = 512                  # query chunk
    NQC = NB // QC            # 8
    NMB = NB // 128           # 32 key blocks per batch
    GRP = 512                 # qkv token group
    NGRP = NB // GRP          # 8  (== NQC, used for interleaving)
    TSL_B = NB // n_cores     # 512-token out-proj slice per core per batch

    sched = _exp_schedule()

    nc = bacc.Bacc("TRN2", target_bir_lowering=False, debug=False,
                   num_devices=n_cores)

    xT_d = nc.declare_dram_parameter("xT", [CB, 128, T], bf16, isOutput=False)
    wT_d = nc.declare_dram_parameter("wT", [3, 128, CB * 128], bf16,
                                     isOutput=False)
    qkvb_d = nc.declare_dram_parameter("qkvb", [3, 128, 1], f32,
                                       isOutput=False)
    owT_d = nc.declare_dram_parameter("owT", [CB, 128, C], bf16,
                                      isOutput=False)
    outb_d = nc.declare_dram_parameter("outb", [1, C], f32, isOutput=False)
    out_d = nc.declare_dram_parameter("out", [B * TSL_B, C], f32,
                                      isOutput=True)

    a2a_in = [nc.dram_tensor(f"a2a_in{b}", [n_cores, 128, TSL_B], bf16)
              for b in range(B)]
    a2a_out = [nc.dram_tensor(f"a2a_out{b}", [n_cores, 128, TSL_B], bf16)
               for b in range(B)]

    with tile.TileContext(nc) as tc:
        with (
            tc.tile_pool(name="persist", bufs=1) as pp,
            tc.tile_pool(name="xt", bufs=2) as xtp,
            tc.tile_pool(name="pexp", bufs=6) as pexpp,
            tc.tile_pool(name="osbp", bufs=6) as osbp,
            tc.tile_pool(name="misc", bufs=2) as mp,
            tc.tile_pool(name="stp", bufs=3, space="PSUM") as stp,
            tc.tile_pool(name="ohp", bufs=2, space="PSUM") as ohp,
        ):
            ident = pp.tile([128, 128], bf16, tag="ident")
            make_identity(nc, ident)

            # ---- persistent weights ----
            wT = pp.tile([128, 3 * CB * 128], bf16, tag="wT")
            wT4 = wT[:].rearrange("p (m cb d) -> p m cb d", m=3, cb=CB)
            for m in range(3):
                nc.sync.dma_start(
                    wT[:, m * CB * 128:(m + 1) * CB * 128], wT_d[m])
            owT = pp.tile([128, CB * C], bf16, tag="owT")
            owT3 = owT[:].rearrange("p (g co) -> p g co", co=C)
            bias_sb = pp.tile([128, 3], f32, tag="bias")
            for m in range(3):
                nc.sync.dma_start(bias_sb[:, m:m + 1], qkvb_d[m])
            outb_f = pp.tile([1, C], f32, tag="outbf")
            nc.sync.dma_start(outb_f[:], outb_d[:])
            outb_sb = pp.tile([1, C], bf16, tag="outb")
            nc.vector.tensor_copy(outb_sb[:], outb_f[:])
            ones_sb = pp.tile([1, 128], bf16, tag="ones")
            nc.vector.memset(ones_sb[:], 1.0)

            # ---- per-batch persistent tensors ----
            qT = [pp.tile([128, NB], bf16, tag=f"qT{b}", name=f"qT{b}")
                  for b in range(B)]
            kT = [pp.tile([128, NB], bf16, tag=f"kT{b}", name=f"kT{b}")
                  for b in range(B)]
            V = [pp.tile([128, NMB * 130], bf16, tag=f"V{b}", name=f"V{b}")
                 for b in range(B)]
            V8 = [pp.tile([128, NMB * 144], f8, tag=f"V8{b}", name=f"V8{b}")
                  for b in range(B)]
            for b in range(B):
                nc.vector.memset(V[b][:], 1.0)
            Oh0 = pp.tile([64, T], bf16, tag="Oh0")
            Oh1 = pp.tile([64, T], bf16, tag="Oh1")
            recv = [pp.tile([128, n_cores * TSL_B], bf16, tag=f"recv{b}",
                             name=f"recv{b}") for b in range(B)]
            # normalization scratch: per (qc mod 4, head) slot
            rcb = pp.tile([1, 4 * QC], bf16, tag="rcb")
            rc = pp.tile([1, 4 * QC], f32, tag="rc")
            rcp = pp.tile([1, 4 * QC], f32, tag="rcp")
            rb = pp.tile([128, 4 * QC], f32, tag="rb")

            def qkv_group(b, grp):
                """qkv projection for 512 tokens of batch b (generator:
                yields between chunks so attention emission can interleave
                finely and keep the exp engines fed)."""
                t0 = b * NB + grp * GRP
                xt = xtp.tile([128, CB * GRP], bf16, tag="xt")
                xt3 = xt[:].rearrange("p (cb t) -> p cb t", t=GRP)
                for cb in range(CB):
                    nc.sync.dma_start(xt3[:, cb], xT_d[cb, :, t0:t0 + GRP])
                yield
                for m in range(3):
                    qp = stp.tile([128, 2 * GRP], f32, tag="st")
                    for cb in range(CB):
                        nc.tensor.matmul(qp[:, 0:GRP], wT4[:, m, cb],
                                         xt3[:, cb],
                                         start=(cb == 0), stop=(cb == CB - 1))
                        if cb == 3:
                            yield
                    if m < 2:
                        dest = (qT if m == 0 else kT)[b][
                            :, grp * GRP:(grp + 1) * GRP]
                        nc.vector.tensor_scalar(dest, qp[:, 0:GRP],
                                                bias_sb[:, m:m + 1],
                                                None, op0=ALU.add)
                        yield
                    else:
                        vs = mp.tile([128, GRP], bf16, tag="vs")
                        nc.vector.tensor_scalar(vs[:], qp[:, 0:GRP],
                                                bias_sb[:, 2:3],
                                                None, op0=ALU.add)
                        tp = stp.tile([128, 2 * GRP], bf16, tag="st")
                        for j in range(GRP // 128):
                            nc.tensor.transpose(
                                tp[:, j * 128:(j + 1) * 128],
                                vs[:, j * 128:(j + 1) * 128], ident[:])
                        mb0 = grp * (GRP // 128)
                        vv = V[b][:].rearrange("p (m d) -> p m d", d=130)
                        tp3 = tp[:, 0:GRP].rearrange("p (j a) -> p j a", a=128)
                        nc.vector.tensor_copy(vv[:, mb0:mb0 + 4, 0:64],
                                              tp3[:, :, 0:64])
                        nc.vector.tensor_copy(vv[:, mb0:mb0 + 4, 65:129],
                                              tp3[:, :, 64:128])
                        v84 = V8[b][:].rearrange("p (m d) -> p m d", d=144)
                        nc.gpsimd.tensor_copy(
                            v84[:, mb0:mb0 + 4, 0:130],
                            vv[:, mb0:mb0 + 4, 0:130])
                        yield

            def attention_qc(b, qc, fins_out, filler=None):
                """S + exp + PV for one 512-query chunk (generator: yields
                after each of the 16 slots).  Deferred normalize closures
                are appended to fins_out.  `filler` is an optional
                generator stepped at every slot."""
                qsl = slice(qc * QC, (qc + 1) * QC)
                oh0 = ohp.tile([65, QC], f32, tag="oh")
                oh1 = ohp.tile([65, QC], f32, tag="oh")
                V83 = V8[b][:].rearrange("p (m d) -> p m d", d=144)

                def s_one(mb):
                    st = stp.tile([128, 2 * QC], f32, tag="st")
                    nc.tensor.matmul(st[:, 0:QC],
                                     kT[b][0:64, mb * 128:mb * 128 + 128],
                                     qT[b][0:64, qsl],
                                     start=True, stop=True)
                    nc.tensor.matmul(st[:, QC:2 * QC],
                                     kT[b][64:128, mb * 128:mb * 128 + 128],
                                     qT[b][64:128, qsl],
                                     start=True, stop=True)
                    return st

                def pv_dr(slot, pe8):
                    """DoubleRow PV covering key blocks 2*slot, 2*slot+1."""
                    pe83 = pe8[:].rearrange("p (m q) -> p m q", q=2 * QC)
                    first = (slot == 0)
                    last = (slot == 15)
                    nc.tensor.matmul(
                        oh0[:], V83[:, 2 * slot:2 * slot + 2, 0:65],
                        pe83[:, :, 0:QC],
                        start=first, stop=last, perf_mode=MPM.DoubleRow)
                    nc.tensor.matmul(
                        oh1[:], V83[:, 2 * slot:2 * slot + 2, 65:130],
                        pe83[:, :, QC:2 * QC],
                        start=first, stop=last, perf_mode=MPM.DoubleRow)

                def pv_bf(mb, pe, slot, half):
                    first = (slot == 0 and half == 0)
                    last = (slot == 15 and half == 1)
                    nc.tensor.matmul(oh0[:],
                                     V[b][:, mb * 130:mb * 130 + 65],
                                     pe[:, 0:QC],
                                     start=first, stop=last)
                    nc.tensor.matmul(oh1[:],
                                     V[b][:, mb * 130 + 65:mb * 130 + 130],
                                     pe[:, QC:2 * QC],
                                     start=first, stop=last)

                def step_filler():
                    if filler is not None:
                        try:
                            next(filler)
                        except StopIteration:
                            pass

                sts = [s_one(0), s_one(1)]
                for slot in range(16):
                    st0, st1 = sts
                    if slot + 1 < 16:
                        sts = [s_one(2 * slot + 2), s_one(2 * slot + 3)]
                    step_filler()
                    if sched[slot] == "act":
                        pe8 = pexpp.tile([128, 4 * QC], f8, tag="pe")
                        nc.scalar.activation(pe8[:, 0:2 * QC], st0[:],
                                             AF.Exp, scale=0.125)
                        nc.scalar.activation(pe8[:, 2 * QC:4 * QC], st1[:],
                                             AF.Exp, scale=0.125)
                        pv_dr(slot, pe8)
                    else:
                        pe0 = pexpp.tile([128, 2 * QC], bf16, tag="pe")
                        pe1 = pexpp.tile([128, 2 * QC], bf16, tag="pe")
                        nc.vector.tensor_scalar(pe0[:].bitcast(i16), st0[:],
                                                BITEXP_A, BITEXP_B,
                                                op0=ALU.mult, op1=ALU.add)
                        nc.vector.tensor_scalar(pe1[:].bitcast(i16), st1[:],
                                                BITEXP_A, BITEXP_B,
                                                op0=ALU.mult, op1=ALU.add)
                        pv_bf(2 * slot, pe0, slot, 0)
                        pv_bf(2 * slot + 1, pe1, slot, 1)
                    yield
                if filler is not None:
                    for _ in filler:
                        pass
                # stage O+den to SBUF immediately (frees the oh PSUM
                # banks); the gpsimd-dependent normalize is returned as a
                # deferred closure so a collective occupying the gpsimd
                # queue cannot stall this qc's PSUM rotation.
                for h, oh in ((0, oh0), (1, oh1)):
                    sl = slice(((qc % 2) * 2 + h) * QC,
                               ((qc % 2) * 2 + h + 1) * QC)
                    osb = osbp.tile([65, QC], bf16, tag="osb")
                    nc.vector.tensor_copy(osb[:], oh[:])
                    nc.sync.dma_start(rcb[0:1, sl], osb[64:65, :])
                    dest = (Oh0 if h == 0 else Oh1)[
                        :, b * NB + qc * QC: b * NB + (qc + 1) * QC]

                    def fin(sl=sl, osb=osb, dest=dest):
                        nc.vector.tensor_copy(rc[0:1, sl], rcb[0:1, sl])
                        nc.vector.reciprocal_approx_fast(rcp[0:1, sl],
                                                         rc[0:1, sl])
                        nc.gpsimd.partition_broadcast(rb[0:64, sl],
                                                      rcp[0:1, sl])
                        nc.vector.scalar_tensor_tensor(
                            dest, osb[0:64, :], 1.0, rb[0:64, sl],
                            op0=ALU.mult, op1=ALU.mult)
                    fins_out.append(fin)

            def a2a_launch(b):
                bsl = slice(b * NB, (b + 1) * NB)
                for j in range(n_cores):
                    nc.sync.dma_start(
                        a2a_in[b][j, 0:64, :],
                        Oh0[:, b * NB + j * TSL_B: b * NB + (j + 1) * TSL_B])
                    nc.sync.dma_start(
                        a2a_in[b][j, 64:128, :],
                        Oh1[:, b * NB + j * TSL_B: b * NB + (j + 1) * TSL_B])
                nc.gpsimd.collective_compute(
                    "AllToAll", ALU.bypass,
                    replica_groups=[list(range(n_cores))],
                    ins=[a2a_in[b].ap().opt()],
                    outs=[a2a_out[b].ap().opt()],
                )
                for g in range(n_cores):
                    nc.sync.dma_start(
                        recv[b][:, g * TSL_B:(g + 1) * TSL_B], a2a_out[b][g])

            def outproj_tb(b, tb):
                """out projection for 128 tokens of my slice of batch b."""
                recv3 = recv[b][:].rearrange("p (g t) -> p g t", t=TSL_B)
                ot = mp.tile([128, C], f32, tag="ot")
                for co2 in range(C // 512):
                    pj = stp.tile([128, 2 * QC], f32, tag="st")
                    for g in range(n_cores):
                        nc.tensor.matmul(
                            pj[:, 0:512],
                            recv3[:, g, tb * 128:tb * 128 + 128],
                            owT3[:, g, co2 * 512:(co2 + 1) * 512],
                            start=(g == 0), stop=False)
                    nc.tensor.matmul(pj[:, 0:512], ones_sb[:],
                                     outb_sb[:, co2 * 512:(co2 + 1) * 512],
                                     start=False, stop=True)
                    nc.vector.tensor_copy(ot[:, co2 * 512:(co2 + 1) * 512],
                                          pj[:, 0:512])
                nc.sync.dma_start(
                    out_d[b * TSL_B + tb * 128: b * TSL_B + (tb + 1) * 128, :],
                    ot[:])

            # ================= pipeline =================
            from itertools import chain

            def drain(g):
                for _ in g:
                    pass

            pending = []

            def flush_pending(n=None):
                k = len(pending) if n is None else n
                for _ in range(k):
                    if pending:
                        pending.pop(0)()

            # qkv(b0) overlapped with attention(b0, qc0): slot s only needs
            # kT/V groups <= (2s+1)//4, so groups 2..7 stream while qc0's
            # early slots already run (keeps Act/DVE fed from the start).
            g0 = [qkv_group(0, g) for g in range(NGRP)]
            drain(g0[0])
            drain(g0[1])
            # out-proj weights are not needed until ~450us in; load them
            # after the first qkv x-tiles so they don't delay PE startup.
            for g in range(CB):
                nc.sync.dma_start(owT3[:, g], owT_d[g])
            att = attention_qc(0, 0, pending)
            done = 0
            for g in range(2, NGRP):
                alive = True
                tick = 0
                while alive:
                    try:
                        next(g0[g])
                    except StopIteration:
                        alive = False
                    tick += 1
                    if tick % 2 == 0 and done < min(2 * g - 1, 16):
                        next(att)
                        done += 1
            drain(att)
            flush_pending(2)
            for qc in range(1, NQC):
                filler = (chain(qkv_group(1, qc - 1), qkv_group(1, NGRP - 1))
                          if qc == NQC - 1 else qkv_group(1, qc - 1))
                drain(attention_qc(0, qc, pending, filler=filler))
                flush_pending(2)
            flush_pending()
            for qc in range(NQC):
                drain(attention_qc(1, qc, pending))
                if qc == 0:
                    a2a_launch(0)
                if qc >= 1:
                    flush_pending(2)
                if 2 <= qc <= 5:
                    outproj_tb(0, qc - 2)
            flush_pending()
            a2a_launch(1)
            for tb in range(TSL_B // 128):
                outproj_tb(1, tb)

    nc.compile()
    return nc


def shard_inputs(x, qkv_w, qkv_b, out_w, out_b, n_cores=8):
    """Per-core input maps with host-side transpose + bf16 cast."""
    import ml_dtypes
    bf = ml_dtypes.bfloat16
    Bv, N, C = x.shape
    T = Bv * N
    CB = C // 128
    # xT [CB, 128, T]
    xT = np.ascontiguousarray(
        x.reshape(T, CB, 128).transpose(1, 2, 0).astype(bf))
    # owT [CB, 128, C]: owT[cb, p, co] = out_w[co, cb*128+p]
    owT = np.ascontiguousarray(
        out_w.astype(bf).T.reshape(CB, 128, C))
    outb = np.ascontiguousarray(out_b.reshape(1, C).astype(np.float32))
    in_maps = []
    for c in range(n_cores):
        r0 = c * 128
        # wT [3, 128, CB*128]: wT[m, p, cb*128+d] = qkv_w[m*C+r0+d, cb*128+p]
        w = np.stack([qkv_w[m * C + r0: m * C + r0 + 128] for m in range(3)])
        wT = np.ascontiguousarray(
            w.astype(bf).reshape(3, 128, CB, 128)
            .transpose(0, 3, 2, 1).reshape(3, 128, CB * 128))
        bvec = np.stack([qkv_b[m * C + r0: m * C + r0 + 128]
                         for m in range(3)])[:, :, None]
        in_maps.append({
            "xT": xT,
            "wT": wT,
            "qkvb": np.ascontiguousarray(bvec.astype(np.float32)),
            "owT": owT,
            "outb": outb,
        })
    return in_maps


def unshard(results, Bv, N, C, n_cores=8):
    """results[c]["out"] is [B*TSL_B, C]: batch-major 512-token slices."""
    TSL_B = N // n_cores
    out = np.empty((Bv, N, C), dtype=np.float32)
    for c in range(n_cores):
        o = results[c]["out"]
        for b in range(Bv):
            out[b, c * TSL_B:(c + 1) * TSL_B, :] = \
                o[b * TSL_B:(b + 1) * TSL_B]
    return out


_NC_CACHE = {}


def kernel(x, qkv_w, qkv_b, out_w, out_b):
    from concourse import bass_utils
    x = np.asarray(x)
    Bv, N, C = x.shape
    key = (N, C)
    if key not in _NC_CACHE:
        _NC_CACHE[key] = build_nc(n_tok_b=N, n_cores=N_CORES, hidden=C)
    nc = _NC_CACHE[key]
    in_maps = shard_inputs(x, np.asarray(qkv_w), np.asarray(qkv_b),
                           np.asarray(out_w), np.asarray(out_b),
                           n_cores=N_CORES)
    res = bass_utils.run_bass_kernel_spmd(nc, in_maps,
                                          core_ids=list(range(N_CORES)))
    return unshard(res.results, Bv, N, C, n_cores=N_CORES)
